# revision 3
# baseline (speedup 1.0000x reference)
"""Trainium2 Bass kernel v2d: nn_MultiHeadAttention_23450521436219.

MultiHeadAttention with softmax over the HEAD axis (dim=1):
  q = Q@Wq.T, k = K@Wk.T, v = V@Wv.T       [B,S,D] -> heads [B,H,S,DK]
  scores = q k^T / sqrt(DK)                 [B,H,Sq,Sk]
  attn = softmax(scores, axis=1)            (over H!)
  out = (attn @ v reshaped) @ Wo.T          [B,S,D]

B=2, S=2048, D=1024, H=16, DK=64.

Sharding: 8 cores = (batch, seq-quarter). Each core computes the output for
its 512 query rows. K/V projections are computed in seq-shards and exchanged
with two fp16 AllGathers (K right after the K projection so its latency
hides under the Q/V projections, V afterwards) over the two 4-core groups.
Softmax over heads is local under query sharding.

Pipeline structure (all engines ~balanced in steady state):
 - per ktile: 16 score matmuls (row-alternating pairs) -> 8 exp calls on
   the scalar engine (the hard floor: 16.8M exps at 1 elem/cycle/lane) ->
   head-sum Z (10 blocks via PE identity-matmul + 6 on DVE) -> 1/Z ->
   broadcast-divide on DVE.
 - ctx accumulates in PSUM per phase (phases of 4,4,4,2,2 ktiles); each
   phase's ctx burst is emitted TWO ktiles later, interleaved into the hp
   slots, so the PE never queues ctx matmuls behind an unfinished divide
   and the exp stream stays fed.
 - V arrives per-phase from DRAM (vslab) to keep SBUF under budget with a
   6-deep E pool.
 - fp16 output, out-projection and DMA per 128-query chunk in the tail.
"""

import os
import sys

sys.path.insert(0, "/opt/trn_rl_repo")

import numpy as np

import concourse.bass as bass
import concourse.mybir as mybir
import concourse.tile as tile
from concourse.vector_clock import ScopedClock
from concourse.bass_utils import run_bass_kernel_spmd

F16 = mybir.dt.float16
F32 = mybir.dt.float32

B, S, D, H, DK = 2, 2048, 1024, 16, 64
NCORES = 8
QSH = 512          # query rows per core
KSH = 512          # key rows per core (shard it projects)
NDC = D // 128     # 8 dout chunks of 128
NKT = S // 128     # 16 key tiles of 128
NHP = H // 2       # 8 head pairs
PE_HEADS = 10      # heads summed via PE identity-matmul; rest via DVE tree
PH_BOUNDS = [(0, 3), (4, 7), (8, 11), (12, 13), (14, 15)]
E_BUFS = 6         # phase depth 4 + 2 (bursts delayed by 2 ktiles)


# ---------------------------------------------------------------------------
# Workaround: this walrus build rejects instructions carrying more than one
# sem wait ("Too many sync wait commands"). After Tile assigns semaphores,
# split every instruction's excess waits onto same-engine nop instructions
# inserted immediately before it.
import bass_rust as _bass_rust

_MAX_WAITS = 1


def _split_excess_waits(nc):
    for fn in nc.m.functions:
        for bb in fn.blocks:
            il = bb.instructions
            i = 0
            while i < len(il):
                inst = il[i]
                si = inst.sync_info
                waits = list(si.on_wait) if si is not None and si.on_wait else []
                if len(waits) > _MAX_WAITS:
                    extra = waits[:-_MAX_WAITS]
                    si.on_wait = waits[-_MAX_WAITS:]
                    for w in extra:
                        nop = nc.engines[inst.engine].nop(nofuse=True)
                        popped = nc.cur_bb.bb.instructions.pop()
                        assert popped is nop.ins
                        popped.sync_info = _bass_rust.SyncInfo(
                            on_wait=[w], on_update=[])
                        il.insert(i, popped)
                        i += 1
                i += 1


def _patched_drain_and_barrier(self, tick_clock, wait_clock):
    nc = self.nc
    probe = nc.sync.nop(nofuse=True)
    wait_clock.add_sem_waits(probe.ins, ScopedClock({None: tick_clock.global_clock}))
    assert self.sems is not None
    popped = nc._tile_sem_poison_stack.pop()
    assert popped is self._sem_poison
    nc.sync.drain()
    nc.all_engine_barrier()
    nc.clear_and_free_semaphores(list(self.sems.allocated().values()))
    nc.all_engine_barrier()
    _split_excess_waits(nc)


tile.TileContext._drain_and_barrier = _patched_drain_and_barrier
# ---------------------------------------------------------------------------

EXP = mybir.ActivationFunctionType.Exp
CPY = mybir.ActivationFunctionType.Copy


def _build(n_reps=1, no_ag=False):
    nc = bass.Bass()

    qt_in = nc.declare_dram_parameter("QT", [D, QSH], F16, isOutput=False)
    kt_in = nc.declare_dram_parameter("KT", [D, KSH], F16, isOutput=False)
    vt_in = nc.declare_dram_parameter("VT", [D, KSH], F16, isOutput=False)
    wqt = nc.declare_dram_parameter("WqT", [D, D], F16, isOutput=False)
    wkt = nc.declare_dram_parameter("WkT", [D, D], F16, isOutput=False)
    wvt = nc.declare_dram_parameter("WvT", [D, D], F16, isOutput=False)
    wot = nc.declare_dram_parameter("WoT", [D, D], F16, isOutput=False)
    idn_in = nc.declare_dram_parameter("IDN", [128, 128], F16, isOutput=False)
    out_d = nc.declare_dram_parameter("out", [QSH, D], F16, isOutput=True)

    for _rep in range(n_reps):
        _build_rep(nc, qt_in, kt_in, vt_in, wqt, wkt, wvt, wot, idn_in,
                   out_d, no_ag)
    return nc


def _build_rep(nc, qt_in, kt_in, vt_in, wqt, wkt, wvt, wot, idn_in, out_d,
               no_ag):
    with tile.TileContext(nc) as tc:
        with (
            tc.tile_pool(name="persist", bufs=1) as pp,
            tc.tile_pool(name="dram", bufs=1, space="DRAM") as dram,
        ):
            # ---- persistent tiles ------------------------------------------
            idn = pp.tile([128, 128], F16, tag="idn")
            wo = [pp.tile([128, D], F16, tag=f"wo{i}", name=f"wo{i}")
                  for i in range(NDC)]
            qpt = [pp.tile([128, QSH], F16, tag=f"qpt{i}", name=f"qpt{i}")
                   for i in range(NDC)]
            ctx32 = [pp.tile([128, QSH], F32, tag=f"ctx32{i}", name=f"ctx32{i}")
                     for i in range(NHP)]

            agk_in = dram.tile([NDC, 128, KSH], F16)
            agk_out = dram.tile([4, NDC, 128, KSH], F16)
            agv_in = dram.tile([NDC, 128, KSH], F16)
            agv_out = dram.tile([4, NDC, 128, KSH], F16)

            # ---- projection phase ------------------------------------------
            with (
                tc.tile_pool(name="proj_in", bufs=1) as pi,
                tc.tile_pool(name="proj_ps", bufs=2, space="PSUM") as pps,
            ):
                wk_t = [pi.tile([128, D], F16, tag=f"wk{i}", name=f"wk{i}")
                        for i in range(NDC)]
                kt_t = [pi.tile([128, KSH], F16, tag=f"kt{i}", name=f"kt{i}")
                        for i in range(NDC)]
                wq_t = [pi.tile([128, D], F16, tag=f"wq{i}", name=f"wq{i}")
                        for i in range(NDC)]
                qt_t = [pi.tile([128, QSH], F16, tag=f"qt{i}", name=f"qt{i}")
                        for i in range(NDC)]
                wv_t = [pi.tile([128, D], F16, tag=f"wv{i}", name=f"wv{i}")
                        for i in range(NDC)]
                vt_t = [pi.tile([128, KSH], F16, tag=f"vt{i}", name=f"vt{i}")
                        for i in range(NDC)]
                # DMA priority order = emission order on the sync queue:
                # K-proj operands first (first half of Wk columns before the
                # second so the dc 0..3 accumulations can start early).
                for i in range(NDC):
                    sl = slice(i * 128, (i + 1) * 128)
                    nc.sync.dma_start(wk_t[i][:, 0:512], wkt[sl, 0:512])
                    nc.sync.dma_start(kt_t[i][:], kt_in[sl, :])
                for i in range(NDC):
                    sl = slice(i * 128, (i + 1) * 128)
                    nc.sync.dma_start(wk_t[i][:, 512:1024], wkt[sl, 512:1024])
                for i in range(NDC):
                    sl = slice(i * 128, (i + 1) * 128)
                    nc.sync.dma_start(wq_t[i][:], wqt[sl, :])
                    nc.sync.dma_start(qt_t[i][:], qt_in[sl, :])
                for i in range(NDC):
                    sl = slice(i * 128, (i + 1) * 128)
                    nc.sync.dma_start(wv_t[i][:], wvt[sl, :])
                    nc.sync.dma_start(vt_t[i][:], vt_in[sl, :])
                nc.sync.dma_start(idn[:], idn_in[:])
                for i in range(NDC):
                    nc.sync.dma_start(wo[i][:], wot[i * 128:(i + 1) * 128, :])

                ksh_t = [pi.tile([128, KSH], F16, tag=f"ksh{i}", name=f"ksh{i}")
                         for i in range(NDC)]
                vsh_t = [pi.tile([128, D], F16, tag=f"vsh{i}", name=f"vsh{i}")
                         for i in range(4)]

                # K projection -> AG1 as early as possible
                for dc in range(NDC):
                    ps = pps.tile([128, KSH], F32, tag="pk")
                    for di in range(NDC):
                        nc.tensor.matmul(
                            ps[:], wk_t[di][:, dc * 128:(dc + 1) * 128],
                            kt_t[di][:],
                            start=(di == 0), stop=(di == NDC - 1))
                    nc.scalar.activation(ksh_t[dc][:], ps[:], CPY)
                    nc.sync.dma_start(agk_in[dc], ksh_t[dc][:])

                if no_ag:
                    for _r in range(4):
                        nc.sync.dma_start(agk_out[_r], agk_in[:])
                else:
                    nc.gpsimd.collective_compute(
                        "AllGather",
                        mybir.AluOpType.bypass,
                        replica_groups=[[0, 1, 2, 3], [4, 5, 6, 7]],
                        ins=[agk_in.opt()],
                        outs=[agk_out.opt()],
                    )

                # Q projection (overlaps AG1)
                for dc in range(NDC):
                    ps = pps.tile([128, QSH], F32, tag="pq")
                    for di in range(NDC):
                        nc.tensor.matmul(
                            ps[:], wq_t[di][:, dc * 128:(dc + 1) * 128],
                            qt_t[di][:],
                            start=(di == 0), stop=(di == NDC - 1))
                    nc.scalar.activation(qpt[dc][:], ps[:], CPY)

                # V projection (overlaps AG1 tail) -> AG2
                for sc in range(4):
                    for nk in range(2):
                        ps = pps.tile([128, 512], F32, tag="pv")
                        for di in range(NDC):
                            nc.tensor.matmul(
                                ps[:], vt_t[di][:, sc * 128:(sc + 1) * 128],
                                wv_t[di][:, nk * 512:(nk + 1) * 512],
                                start=(di == 0), stop=(di == NDC - 1))
                        nc.scalar.activation(
                            vsh_t[sc][:, nk * 512:(nk + 1) * 512], ps[:], CPY)
                    nc.sync.dma_start(agv_in[2 * sc], vsh_t[sc][:, 0:512])
                    nc.sync.dma_start(agv_in[2 * sc + 1], vsh_t[sc][:, 512:1024])

                if no_ag:
                    for _r in range(4):
                        nc.sync.dma_start(agv_out[_r], agv_in[:])
                else:
                    nc.gpsimd.collective_compute(
                        "AllGather",
                        mybir.AluOpType.bypass,
                        replica_groups=[[0, 1, 2, 3], [4, 5, 6, 7]],
                        ins=[agv_in.opt()],
                        outs=[agv_out.opt()],
                    )

            # ---- attention phase -------------------------------------------
            with (
                tc.tile_pool(name="attn_sb", bufs=1) as pa,
                tc.tile_pool(name="vslab_p", bufs=8) as pv,
                tc.tile_pool(name="attn_dve", bufs=2) as pd,
                tc.tile_pool(name="E_pool", bufs=E_BUFS) as pe,
                tc.tile_pool(name="ps_s", bufs=2, space="PSUM") as ps_s,
                tc.tile_pool(name="ps_d", bufs=2, space="PSUM") as ps_d,
                tc.tile_pool(name="ps_c", bufs=2, space="PSUM") as ps_c,
            ):
                # K^T assembled fully resident, loaded hp-major so scores for
                # early head-pairs can start as soon as their rows land.
                ktf = [pa.tile([128, S], F16, tag=f"ktf{i}", name=f"ktf{i}")
                       for i in range(NDC)]
                for dc in range(NDC):
                    for r in range(4):
                        nc.sync.dma_start(
                            ktf[dc][:, r * KSH:(r + 1) * KSH], agk_out[r, dc])

                vslab = {}

                def load_vslab_phase(ph0, ph1):
                    for kt2 in range(ph0, ph1 + 1):
                        v = pv.tile([128, D], F16, tag="vs", name="vs")
                        vslab[kt2] = v
                        r, sub = kt2 // 4, kt2 % 4
                        nc.sync.dma_start(v[:, 0:512], agv_out[r, 2 * sub])
                        nc.sync.dma_start(v[:, 512:1024], agv_out[r, 2 * sub + 1])

                E_tiles = {}

                def emit_ctx_hp(hp, ph0, ph1):
                    """ctx accumulation for head-pair hp over ktiles
                    [ph0..ph1] into a PSUM tile, then fold into ctx32."""
                    n = ph1 - ph0 + 1
                    cps = ps_c.tile([128, QSH], F32, tag="c")
                    for i, k2 in enumerate(range(ph0, ph1 + 1)):
                        Ek = E_tiles[k2]
                        ha, hb = 2 * hp, 2 * hp + 1
                        nc.tensor.matmul(
                            cps[0:64, :], vslab[k2][:, ha * 64:(ha + 1) * 64],
                            Ek[:, ha * 512:(ha + 1) * 512],
                            start=(i == 0), stop=(i == n - 1),
                            tile_position=(0, 0))
                        nc.tensor.matmul(
                            cps[64:128, :], vslab[k2][:, hb * 64:(hb + 1) * 64],
                            Ek[:, hb * 512:(hb + 1) * 512],
                            start=(i == 0), stop=(i == n - 1),
                            tile_position=(0, 64))
                    if ph0 == 0:
                        nc.vector.tensor_copy(ctx32[hp][:], cps[:])
                    else:
                        nc.vector.tensor_add(ctx32[hp][:], cps[:], ctx32[hp][:])

                ph_starts = {s: (s, e) for (s, e) in PH_BOUNDS}
                # bursts are hosted two ktiles after their phase end
                burst_at = {}
                tail_bursts = []
                for (s, e) in PH_BOUNDS:
                    if e + 2 < NKT:
                        burst_at[e + 2] = (s, e)
                    else:
                        tail_bursts.append((s, e))

                for kt in range(NKT):
                    if kt in ph_starts:
                        load_vslab_phase(*ph_starts[kt])
                    E = pe.tile([128, H * 512], F16, tag="E")
                    E_tiles[kt] = E
                    pending = burst_at.get(kt)
                    # scores + exp per head pair; a delayed ctx burst's
                    # matmuls interleave into the hp slots
                    for hp in range(NHP):
                        sc_ps = ps_s.tile([128, 1024], F32, tag="s")
                        kcols = slice(kt * 128, (kt + 1) * 128)
                        nc.tensor.matmul(
                            sc_ps[:, 0:512],
                            ktf[hp][0:64, kcols], qpt[hp][0:64, :],
                            start=True, stop=True)
                        nc.tensor.matmul(
                            sc_ps[:, 512:1024],
                            ktf[hp][64:128, kcols], qpt[hp][64:128, :],
                            start=True, stop=True)
                        nc.scalar.activation(
                            E[:, hp * 1024:(hp + 1) * 1024], sc_ps[:],
                            EXP, scale=0.125)
                        if pending is not None:
                            emit_ctx_hp(hp, *pending)

                    # head-sum Z: PE identity-matmuls + DVE tree for the rest
                    dps = ps_d.tile([128, 512], F32, tag="dps")
                    for j in range(PE_HEADS):
                        nc.tensor.matmul(
                            dps[:], idn[:], E[:, j * 512:(j + 1) * 512],
                            start=(j == 0), stop=(j == PE_HEADS - 1))
                    t0 = pd.tile([128, 3 * 512], F16, tag="t0")
                    nc.vector.tensor_add(
                        t0[:], E[:, 10 * 512:13 * 512], E[:, 13 * 512:16 * 512])
                    t2 = pd.tile([128, 512], F16, tag="t2")
                    nc.vector.tensor_add(t2[:], t0[:, 0:512], t0[:, 512:1024])
                    nc.vector.tensor_add(t2[:], t2[:], t0[:, 1024:1536])
                    dfull = pd.tile([128, 512], F32, tag="dfull")
                    nc.vector.tensor_add(dfull[:], dps[:], t2[:])
                    dinv = pd.tile([128, 512], F32, tag="dinv")
                    nc.vector.reciprocal(dinv[:], dfull[:])
                    dinv16 = pd.tile([128, 512], F16, tag="dinv16")
                    nc.vector.tensor_copy(dinv16[:], dinv[:])
                    # divide: broadcast muls, 4 head-blocks per instruction
                    d3 = dinv16[:].unsqueeze(1).broadcast_to([128, 4, 512])
                    for g in range(4):
                        e3 = E[:, g * 2048:(g + 1) * 2048].rearrange(
                            "p (h q) -> p h q", h=4)
                        nc.vector.tensor_mul(e3, e3, d3)

                # ---- tail: remaining ctx bursts, ctx16 cvt, out proj -------
                for (s, e) in tail_bursts:
                    for hp in range(NHP):
                        emit_ctx_hp(hp, s, e)

                with (
                    tc.tile_pool(name="out_sb", bufs=2) as po,
                    tc.tile_pool(name="out_ps", bufs=4, space="PSUM") as pos,
                ):
                    ctx16 = [po.tile([128, QSH], F16, tag=f"c16_{i}",
                                     name=f"c16_{i}") for i in range(NHP)]
                    for hp in range(NHP):
                        nc.vector.tensor_copy(ctx16[hp][:], ctx32[hp][:])
                    for qc in range(QSH // 128):
                        ost = po.tile([128, D], F16, tag="ost")
                        for nk in range(2):
                            ps = pos.tile([128, 512], F32, tag="po")
                            for di in range(NDC):
                                nc.tensor.matmul(
                                    ps[:], ctx16[di][:, qc * 128:(qc + 1) * 128],
                                    wo[di][:, nk * 512:(nk + 1) * 512],
                                    start=(di == 0), stop=(di == NDC - 1))
                            nc.scalar.activation(
                                ost[:, nk * 512:(nk + 1) * 512], ps[:], CPY)
                        nc.sync.dma_start(out_d[qc * 128:(qc + 1) * 128, :], ost[:])


_NC_CACHE = None


def kernel(Query, Key, Value, Wq, Wk, Wv, Wo):
    global _NC_CACHE
    if _NC_CACHE is None:
        _NC_CACHE = _build()
    nc = _NC_CACHE

    Query = np.asarray(Query, np.float32)
    Key = np.asarray(Key, np.float32)
    Value = np.asarray(Value, np.float32)
    wq_t = np.ascontiguousarray(np.asarray(Wq, np.float32).T.astype(np.float16))
    wk_t = np.ascontiguousarray(np.asarray(Wk, np.float32).T.astype(np.float16))
    wv_t = np.ascontiguousarray(np.asarray(Wv, np.float32).T.astype(np.float16))
    wo_t = np.ascontiguousarray(np.asarray(Wo, np.float32).T.astype(np.float16))
    idn = np.eye(128, dtype=np.float16)

    in_maps = []
    for c in range(NCORES):
        b, r = c // 4, c % 4
        rows = slice(r * QSH, (r + 1) * QSH)
        in_maps.append({
            "QT": np.ascontiguousarray(Query[b, rows, :].T.astype(np.float16)),
            "KT": np.ascontiguousarray(Key[b, rows, :].T.astype(np.float16)),
            "VT": np.ascontiguousarray(Value[b, rows, :].T.astype(np.float16)),
            "WqT": wq_t, "WkT": wk_t, "WvT": wv_t, "WoT": wo_t,
            "IDN": idn,
        })

    global _last_in_maps
    _last_in_maps = in_maps
    res = run_bass_kernel_spmd(nc, in_maps, core_ids=list(range(NCORES)))

    out = np.empty((B, S, D), np.float32)
    for c in range(NCORES):
        b, r = c // 4, c % 4
        out[b, r * QSH:(r + 1) * QSH, :] = res.results[c]["out"].astype(np.float32)
    return out


# revision 7
# speedup vs baseline: 1.4564x; 1.4564x over previous
"""Trainium2 Bass kernel v2d: nn_MultiHeadAttention_23450521436219.

MultiHeadAttention with softmax over the HEAD axis (dim=1):
  q = Q@Wq.T, k = K@Wk.T, v = V@Wv.T       [B,S,D] -> heads [B,H,S,DK]
  scores = q k^T / sqrt(DK)                 [B,H,Sq,Sk]
  attn = softmax(scores, axis=1)            (over H!)
  out = (attn @ v reshaped) @ Wo.T          [B,S,D]

B=2, S=2048, D=1024, H=16, DK=64.

Sharding: 8 cores = (batch, seq-quarter). Each core computes the output for
its 512 query rows. K/V projections are computed in seq-shards and exchanged
with two fp16 AllGathers (K right after the K projection so its latency
hides under the Q/V projections, V afterwards) over the two 4-core groups.
Softmax over heads is local under query sharding.

Pipeline structure (all engines ~balanced in steady state):
 - per ktile: 16 score matmuls (row-alternating pairs) -> 8 exp calls on
   the scalar engine (the hard floor: 16.8M exps at 1 elem/cycle/lane) ->
   head-sum Z (10 blocks via PE identity-matmul + 6 on DVE) -> 1/Z ->
   broadcast-divide on DVE.
 - ctx accumulates in PSUM per phase (phases of 4,4,4,2,2 ktiles); each
   phase's ctx burst is emitted TWO ktiles later, interleaved into the hp
   slots, so the PE never queues ctx matmuls behind an unfinished divide
   and the exp stream stays fed.
 - V arrives per-phase from DRAM (vslab) to keep SBUF under budget with a
   6-deep E pool.
 - fp16 output, out-projection and DMA per 128-query chunk in the tail.
"""

import os
import sys

sys.path.insert(0, "/opt/trn_rl_repo")

import numpy as np

import concourse.bass as bass
import concourse.mybir as mybir
import concourse.tile as tile
from concourse.vector_clock import ScopedClock
from concourse.bass_utils import run_bass_kernel_spmd

F16 = mybir.dt.float16
F32 = mybir.dt.float32

B, S, D, H, DK = 2, 2048, 1024, 16, 64
NCORES = 8
QSH = 512          # query rows per core
KSH = 512          # key rows per core (shard it projects)
NDC = D // 128     # 8 dout chunks of 128
NKT = S // 128     # 16 key tiles of 128
NHP = H // 2       # 8 head pairs
PE_HEADS = 10      # heads summed via PE identity-matmul; rest via DVE tree
PH_BOUNDS = [(0, 3), (4, 7), (8, 11), (12, 13), (14, 15)]
E_BUFS = 6         # phase depth 4 + 2 (bursts delayed by 2 ktiles)


# ---------------------------------------------------------------------------
# Workaround: this walrus build rejects instructions carrying more than one
# sem wait ("Too many sync wait commands"). After Tile assigns semaphores,
# split every instruction's excess waits onto same-engine nop instructions
# inserted immediately before it.
import bass_rust as _bass_rust

_MAX_WAITS = 1


def _split_excess_waits(nc):
    for fn in nc.m.functions:
        for bb in fn.blocks:
            il = bb.instructions
            i = 0
            while i < len(il):
                inst = il[i]
                si = inst.sync_info
                waits = list(si.on_wait) if si is not None and si.on_wait else []
                if len(waits) > _MAX_WAITS:
                    extra = waits[:-_MAX_WAITS]
                    si.on_wait = waits[-_MAX_WAITS:]
                    for w in extra:
                        nop = nc.engines[inst.engine].nop(nofuse=True)
                        popped = nc.cur_bb.bb.instructions.pop()
                        assert popped is nop.ins
                        popped.sync_info = _bass_rust.SyncInfo(
                            on_wait=[w], on_update=[])
                        il.insert(i, popped)
                        i += 1
                i += 1


def _patched_drain_and_barrier(self, tick_clock, wait_clock):
    nc = self.nc
    probe = nc.sync.nop(nofuse=True)
    wait_clock.add_sem_waits(probe.ins, ScopedClock({None: tick_clock.global_clock}))
    assert self.sems is not None
    popped = nc._tile_sem_poison_stack.pop()
    assert popped is self._sem_poison
    nc.sync.drain()
    nc.all_engine_barrier()
    nc.clear_and_free_semaphores(list(self.sems.allocated().values()))
    nc.all_engine_barrier()
    _split_excess_waits(nc)


tile.TileContext._drain_and_barrier = _patched_drain_and_barrier
# ---------------------------------------------------------------------------

EXP = mybir.ActivationFunctionType.Exp
CPY = mybir.ActivationFunctionType.Copy


def _build(n_reps=1, no_ag=False):
    nc = bass.Bass()

    qt_in = nc.declare_dram_parameter("QT", [D, QSH], F16, isOutput=False)
    kt_in = nc.declare_dram_parameter("KT", [D, KSH], F16, isOutput=False)
    vt_in = nc.declare_dram_parameter("VT", [D, KSH], F16, isOutput=False)
    wqt = nc.declare_dram_parameter("WqT", [D, D], F16, isOutput=False)
    wkt = nc.declare_dram_parameter("WkT", [D, D], F16, isOutput=False)
    wvt = nc.declare_dram_parameter("WvT", [D, D], F16, isOutput=False)
    wot = nc.declare_dram_parameter("WoT", [D, D], F16, isOutput=False)
    idn_in = nc.declare_dram_parameter("IDN", [128, 128], F16, isOutput=False)
    out_d = nc.declare_dram_parameter("out", [QSH, D], F16, isOutput=True)

    for _rep in range(n_reps):
        _build_rep(nc, qt_in, kt_in, vt_in, wqt, wkt, wvt, wot, idn_in,
                   out_d, no_ag)
    return nc


def _build_rep(nc, qt_in, kt_in, vt_in, wqt, wkt, wvt, wot, idn_in, out_d,
               no_ag):
    with tile.TileContext(nc) as tc:
        with (
            tc.tile_pool(name="persist", bufs=1) as pp,
            tc.tile_pool(name="dram", bufs=1, space="DRAM") as dram,
        ):
            # ---- persistent tiles ------------------------------------------
            idn = pp.tile([128, 128], F16, tag="idn")
            wo = [pp.tile([128, D], F16, tag=f"wo{i}", name=f"wo{i}")
                  for i in range(NDC)]
            qpt = [pp.tile([128, QSH], F16, tag=f"qpt{i}", name=f"qpt{i}")
                   for i in range(NDC)]
            ctx32 = [pp.tile([128, QSH], F32, tag=f"ctx32{i}", name=f"ctx32{i}")
                     for i in range(NHP)]

            agk_in = dram.tile([NDC, 128, KSH], F16)
            agk_out = dram.tile([4, NDC, 128, KSH], F16)
            agv_in = dram.tile([NDC, 128, KSH], F16)
            agv_out = dram.tile([4, NDC, 128, KSH], F16)

            # ---- projection phase ------------------------------------------
            with (
                tc.tile_pool(name="proj_in", bufs=1) as pi,
                tc.tile_pool(name="proj_ps", bufs=2, space="PSUM") as pps,
            ):
                wk_t = [pi.tile([128, D], F16, tag=f"wk{i}", name=f"wk{i}")
                        for i in range(NDC)]
                kt_t = [pi.tile([128, KSH], F16, tag=f"kt{i}", name=f"kt{i}")
                        for i in range(NDC)]
                wq_t = [pi.tile([128, D], F16, tag=f"wq{i}", name=f"wq{i}")
                        for i in range(NDC)]
                qt_t = [pi.tile([128, QSH], F16, tag=f"qt{i}", name=f"qt{i}")
                        for i in range(NDC)]
                wv_t = [pi.tile([128, D], F16, tag=f"wv{i}", name=f"wv{i}")
                        for i in range(NDC)]
                vt_t = [pi.tile([128, KSH], F16, tag=f"vt{i}", name=f"vt{i}")
                        for i in range(NDC)]
                # DMA priority order = emission order on the sync queue:
                # K-proj operands first (first half of Wk columns before the
                # second so the dc 0..3 accumulations can start early).
                for i in range(NDC):
                    sl = slice(i * 128, (i + 1) * 128)
                    nc.sync.dma_start(wk_t[i][:, 0:512], wkt[sl, 0:512])
                    nc.sync.dma_start(kt_t[i][:], kt_in[sl, :])
                for i in range(NDC):
                    sl = slice(i * 128, (i + 1) * 128)
                    nc.sync.dma_start(wk_t[i][:, 512:1024], wkt[sl, 512:1024])
                for i in range(NDC):
                    sl = slice(i * 128, (i + 1) * 128)
                    nc.sync.dma_start(wq_t[i][:], wqt[sl, :])
                    nc.sync.dma_start(qt_t[i][:], qt_in[sl, :])
                for i in range(NDC):
                    sl = slice(i * 128, (i + 1) * 128)
                    nc.sync.dma_start(wv_t[i][:], wvt[sl, :])
                    nc.sync.dma_start(vt_t[i][:], vt_in[sl, :])
                nc.sync.dma_start(idn[:], idn_in[:])
                for i in range(NDC):
                    nc.sync.dma_start(wo[i][:], wot[i * 128:(i + 1) * 128, :])

                ksh_t = [pi.tile([128, KSH], F16, tag=f"ksh{i}", name=f"ksh{i}")
                         for i in range(NDC)]
                vsh_t = [pi.tile([128, D], F16, tag=f"vsh{i}", name=f"vsh{i}")
                         for i in range(4)]

                # K projection -> AG1 as early as possible
                for dc in range(NDC):
                    ps = pps.tile([128, KSH], F32, tag="pk")
                    for di in range(NDC):
                        nc.tensor.matmul(
                            ps[:], wk_t[di][:, dc * 128:(dc + 1) * 128],
                            kt_t[di][:],
                            start=(di == 0), stop=(di == NDC - 1))
                    nc.scalar.activation(ksh_t[dc][:], ps[:], CPY)
                    nc.sync.dma_start(agk_in[dc], ksh_t[dc][:])

                if no_ag:
                    for _r in range(4):
                        nc.sync.dma_start(agk_out[_r], agk_in[:])
                else:
                    nc.gpsimd.collective_compute(
                        "AllGather",
                        mybir.AluOpType.bypass,
                        replica_groups=[[0, 1, 2, 3], [4, 5, 6, 7]],
                        ins=[agk_in.opt()],
                        outs=[agk_out.opt()],
                    )

                # Q projection (overlaps AG1)
                for dc in range(NDC):
                    ps = pps.tile([128, QSH], F32, tag="pq")
                    for di in range(NDC):
                        nc.tensor.matmul(
                            ps[:], wq_t[di][:, dc * 128:(dc + 1) * 128],
                            qt_t[di][:],
                            start=(di == 0), stop=(di == NDC - 1))
                    nc.scalar.activation(qpt[dc][:], ps[:], CPY)

                # V projection (overlaps AG1 tail) -> AG2
                for sc in range(4):
                    for nk in range(2):
                        ps = pps.tile([128, 512], F32, tag="pv")
                        for di in range(NDC):
                            nc.tensor.matmul(
                                ps[:], vt_t[di][:, sc * 128:(sc + 1) * 128],
                                wv_t[di][:, nk * 512:(nk + 1) * 512],
                                start=(di == 0), stop=(di == NDC - 1))
                        nc.scalar.activation(
                            vsh_t[sc][:, nk * 512:(nk + 1) * 512], ps[:], CPY)
                    nc.sync.dma_start(agv_in[2 * sc], vsh_t[sc][:, 0:512])
                    nc.sync.dma_start(agv_in[2 * sc + 1], vsh_t[sc][:, 512:1024])

                if no_ag:
                    for _r in range(4):
                        nc.sync.dma_start(agv_out[_r], agv_in[:])
                else:
                    nc.gpsimd.collective_compute(
                        "AllGather",
                        mybir.AluOpType.bypass,
                        replica_groups=[[0, 1, 2, 3], [4, 5, 6, 7]],
                        ins=[agv_in.opt()],
                        outs=[agv_out.opt()],
                    )

            # ---- attention phase -------------------------------------------
            with (
                tc.tile_pool(name="attn_sb", bufs=1) as pa,
                tc.tile_pool(name="vslab_p", bufs=8) as pv,
                tc.tile_pool(name="attn_dve", bufs=2) as pd,
                tc.tile_pool(name="E_pool", bufs=E_BUFS) as pe,
                tc.tile_pool(name="ps_s", bufs=2, space="PSUM") as ps_s,
                tc.tile_pool(name="ps_d", bufs=2, space="PSUM") as ps_d,
                tc.tile_pool(name="ps_c", bufs=2, space="PSUM") as ps_c,
            ):
                # K^T assembled fully resident, loaded hp-major so scores for
                # early head-pairs can start as soon as their rows land.
                ktf = [pa.tile([128, S], F16, tag=f"ktf{i}", name=f"ktf{i}")
                       for i in range(NDC)]
                for dc in range(NDC):
                    for r in range(4):
                        nc.sync.dma_start(
                            ktf[dc][:, r * KSH:(r + 1) * KSH], agk_out[r, dc])

                vslab = {}

                def load_vslab_phase(ph0, ph1):
                    for kt2 in range(ph0, ph1 + 1):
                        v = pv.tile([128, D], F16, tag="vs", name="vs")
                        vslab[kt2] = v
                        r, sub = kt2 // 4, kt2 % 4
                        nc.sync.dma_start(v[:, 0:512], agv_out[r, 2 * sub])
                        nc.sync.dma_start(v[:, 512:1024], agv_out[r, 2 * sub + 1])

                E_tiles = {}

                def emit_ctx_hp(hp, ph0, ph1):
                    """ctx accumulation for head-pair hp over ktiles
                    [ph0..ph1] into a PSUM tile, then fold into ctx32."""
                    n = ph1 - ph0 + 1
                    cps = ps_c.tile([128, QSH], F32, tag="c")
                    for i, k2 in enumerate(range(ph0, ph1 + 1)):
                        Ek = E_tiles[k2]
                        ha, hb = 2 * hp, 2 * hp + 1
                        nc.tensor.matmul(
                            cps[0:64, :], vslab[k2][:, ha * 64:(ha + 1) * 64],
                            Ek[:, ha * 512:(ha + 1) * 512],
                            start=(i == 0), stop=(i == n - 1),
                            tile_position=(0, 0))
                        nc.tensor.matmul(
                            cps[64:128, :], vslab[k2][:, hb * 64:(hb + 1) * 64],
                            Ek[:, hb * 512:(hb + 1) * 512],
                            start=(i == 0), stop=(i == n - 1),
                            tile_position=(0, 64))
                    if ph0 == 0:
                        nc.vector.tensor_copy(ctx32[hp][:], cps[:])
                    else:
                        nc.vector.tensor_add(ctx32[hp][:], cps[:], ctx32[hp][:])

                ph_starts = {s: (s, e) for (s, e) in PH_BOUNDS}
                # bursts are hosted two ktiles after their phase end
                burst_at = {}
                tail_bursts = []
                for (s, e) in PH_BOUNDS:
                    if e + 2 < NKT:
                        burst_at[e + 2] = (s, e)
                    else:
                        tail_bursts.append((s, e))

                for kt in range(NKT):
                    if kt in ph_starts:
                        load_vslab_phase(*ph_starts[kt])
                    E = pe.tile([128, H * 512], F16, tag="E")
                    E_tiles[kt] = E
                    pending = burst_at.get(kt)
                    # scores + exp per head pair; a delayed ctx burst's
                    # matmuls interleave into the hp slots
                    for hp in range(NHP):
                        sc_ps = ps_s.tile([128, 1024], F32, tag="s")
                        kcols = slice(kt * 128, (kt + 1) * 128)
                        nc.tensor.matmul(
                            sc_ps[:, 0:512],
                            ktf[hp][0:64, kcols], qpt[hp][0:64, :],
                            start=True, stop=True)
                        nc.tensor.matmul(
                            sc_ps[:, 512:1024],
                            ktf[hp][64:128, kcols], qpt[hp][64:128, :],
                            start=True, stop=True)
                        nc.scalar.activation(
                            E[:, hp * 1024:(hp + 1) * 1024], sc_ps[:],
                            EXP, scale=0.125)
                        if pending is not None:
                            emit_ctx_hp(hp, *pending)

                    # head-sum Z: PE identity-matmuls + DVE tree for the rest
                    dps = ps_d.tile([128, 512], F32, tag="dps")
                    for j in range(PE_HEADS):
                        nc.tensor.matmul(
                            dps[:], idn[:], E[:, j * 512:(j + 1) * 512],
                            start=(j == 0), stop=(j == PE_HEADS - 1))
                    t0 = pd.tile([128, 3 * 512], F16, tag="t0")
                    nc.vector.tensor_add(
                        t0[:], E[:, 10 * 512:13 * 512], E[:, 13 * 512:16 * 512])
                    t2 = pd.tile([128, 512], F16, tag="t2")
                    nc.vector.tensor_add(t2[:], t0[:, 0:512], t0[:, 512:1024])
                    nc.vector.tensor_add(t2[:], t2[:], t0[:, 1024:1536])
                    dfull = pd.tile([128, 512], F32, tag="dfull")
                    nc.vector.tensor_add(dfull[:], dps[:], t2[:])
                    dinv = pd.tile([128, 512], F32, tag="dinv")
                    nc.vector.reciprocal(dinv[:], dfull[:])
                    dinv16 = pd.tile([128, 512], F16, tag="dinv16")
                    nc.vector.tensor_copy(dinv16[:], dinv[:])
                    # divide: broadcast muls, 4 head-blocks per instruction
                    d3 = dinv16[:].unsqueeze(1).broadcast_to([128, 4, 512])
                    for g in range(4):
                        e3 = E[:, g * 2048:(g + 1) * 2048].rearrange(
                            "p (h q) -> p h q", h=4)
                        nc.vector.tensor_mul(e3, e3, d3)

                # ---- tail: remaining ctx bursts, ctx16 cvt, out proj -------
                for (s, e) in tail_bursts:
                    for hp in range(NHP):
                        emit_ctx_hp(hp, s, e)

                with (
                    tc.tile_pool(name="out_sb", bufs=2) as po,
                    tc.tile_pool(name="out_ps", bufs=4, space="PSUM") as pos,
                ):
                    ctx16 = [po.tile([128, QSH], F16, tag=f"c16_{i}",
                                     name=f"c16_{i}") for i in range(NHP)]
                    for hp in range(NHP):
                        nc.vector.tensor_copy(ctx16[hp][:], ctx32[hp][:])
                    for qc in range(QSH // 128):
                        ost = po.tile([128, D], F16, tag="ost")
                        for nk in range(2):
                            ps = pos.tile([128, 512], F32, tag="po")
                            for di in range(NDC):
                                nc.tensor.matmul(
                                    ps[:], ctx16[di][:, qc * 128:(qc + 1) * 128],
                                    wo[di][:, nk * 512:(nk + 1) * 512],
                                    start=(di == 0), stop=(di == NDC - 1))
                            nc.scalar.activation(
                                ost[:, nk * 512:(nk + 1) * 512], ps[:], CPY)
                        nc.sync.dma_start(out_d[qc * 128:(qc + 1) * 128, :], ost[:])


_NC_CACHE = None


def kernel(Query, Key, Value, Wq, Wk, Wv, Wo):
    global _NC_CACHE
    if _NC_CACHE is None:
        _NC_CACHE = _build()
    nc = _NC_CACHE

    Query = np.asarray(Query, np.float32)
    Key = np.asarray(Key, np.float32)
    Value = np.asarray(Value, np.float32)
    wq_t = np.ascontiguousarray(np.asarray(Wq, np.float32).T.astype(np.float16))
    wk_t = np.ascontiguousarray(np.asarray(Wk, np.float32).T.astype(np.float16))
    wv_t = np.ascontiguousarray(np.asarray(Wv, np.float32).T.astype(np.float16))
    wo_t = np.ascontiguousarray(np.asarray(Wo, np.float32).T.astype(np.float16))
    idn = np.eye(128, dtype=np.float16)

    in_maps = []
    for c in range(NCORES):
        b, r = c // 4, c % 4
        rows = slice(r * QSH, (r + 1) * QSH)
        in_maps.append({
            "QT": np.ascontiguousarray(Query[b, rows, :].T.astype(np.float16)),
            "KT": np.ascontiguousarray(Key[b, rows, :].T.astype(np.float16)),
            "VT": np.ascontiguousarray(Value[b, rows, :].T.astype(np.float16)),
            "WqT": wq_t, "WkT": wk_t, "WvT": wv_t, "WoT": wo_t,
            "IDN": idn,
        })

    global _last_in_maps
    _last_in_maps = in_maps
    res = run_bass_kernel_spmd(nc, in_maps, core_ids=list(range(NCORES)))

    out = np.empty((B, S, D), np.float32)
    for c in range(NCORES):
        b, r = c // 4, c % 4
        out[b, r * QSH:(r + 1) * QSH, :] = res.results[c]["out"].astype(np.float32)
    return out
